# revision 7
# baseline (speedup 1.0000x reference)
"""GNN message-passing kernel for 8 Trainium2 NeuronCores.

Computes: relu(concat([x @ Wx + bx, segment_sum(edge_attr, src) @ We + be], axis=1))

Strategy (graph-parallel, per the sharding hint):
  - Nodes are sharded 8 ways (12500 per core); edges are bucketed by the
    core that owns their source node, so the segment-sum is core-local.
  - On-device segment-sum: edges are host-bucketed into 64-node windows;
    each 128-edge chunk contributes via a one-hot matmul on the PE
    (stationary = edge_attr chunk [128,32] bf16, moving = one-hot
    [128,64] bf16 built on DVE with is_equal(iota, idx)), accumulating
    aggT [32, nodes] in PSUM.
  - Epilogue per 128 nodes: two fp32r matmuls with N=256 fold both
    linears, the bias add and the concat into one PSUM tile
    ([Wx | 0] and [[0 | We]; [bx | be]]); ACT applies relu from PSUM.
"""

import sys

sys.path.insert(0, "/opt/trn_rl_repo")

import numpy as np
import ml_dtypes

from concourse import mybir, bacc
import concourse.tile as tile
from concourse.bass_utils import run_bass_kernel_spmd

# Problem constants (hardcoded per the nn_NodeCentric spec)
N = 100_000
E = 1_600_000
FE = 32
FX = 128
OX = 128
OE = 128
NCORES = 8
NPC = N // NCORES          # nodes per core = 12500
W = 64                     # node window (one-hot moving width)
SUP = 512                  # supertile = one PSUM bank of aggT columns
NSUP = (NPC + SUP - 1) // SUP          # 25
NPC_PAD = NSUP * SUP                   # 12800
WIN_PER_CORE = NPC_PAD // W            # 200
WPS = SUP // W                         # 8 windows per supertile

BF16 = ml_dtypes.bfloat16

_program_cache: dict[int, object] = {}
last_results = None  # BassKernelResults of the most recent run (for test harness)


def _build_program(K: int):
    """Build the (identical-across-cores) Bass program for K chunks/window."""
    C = WIN_PER_CORE * K        # chunks per core
    CPS = WPS * K               # chunks per supertile

    nc = bacc.Bacc("TRN2", target_bir_lowering=False, debug=False)
    f32 = mybir.dt.float32
    f32r = mybir.dt.float32r
    bf16 = mybir.dt.bfloat16

    d_attr = nc.dram_tensor("attr", [128, C * 32], bf16, kind="ExternalInput")
    d_idx = nc.dram_tensor("idx", [128, C], f32, kind="ExternalInput")
    d_xt = nc.dram_tensor("xt", [128, NPC_PAD], f32, kind="ExternalInput")
    d_wx0 = nc.dram_tensor("wx0", [128, 256], f32, kind="ExternalInput")
    d_webb = nc.dram_tensor("webb", [33, 256], f32, kind="ExternalInput")
    d_iota = nc.dram_tensor("iota", [128, W], bf16, kind="ExternalInput")
    d_out = nc.dram_tensor("out", [NPC_PAD, 256], f32, kind="ExternalOutput")

    with tile.TileContext(nc) as tc:
        with (
            tc.tile_pool(name="const", bufs=1) as constp,
            tc.tile_pool(name="attr", bufs=3) as attrp,
            tc.tile_pool(name="oh", bufs=8) as ohp,
            tc.tile_pool(name="agg", bufs=2) as aggp,
            tc.tile_pool(name="outs", bufs=3) as outp,
            tc.tile_pool(name="psagg", bufs=2, space="PSUM") as psaggp,
            tc.tile_pool(name="psout", bufs=2, space="PSUM") as psoutp,
        ):
            t_idx = constp.tile([128, C], f32)
            nc.sync.dma_start(out=t_idx[:], in_=d_idx[:])
            t_xt_raw = constp.tile([128, NPC_PAD], f32)
            nc.sync.dma_start(out=t_xt_raw[:], in_=d_xt[:])
            t_xt = constp.tile([128, NPC_PAD], f32r)
            nc.vector.tensor_copy(out=t_xt[:], in_=t_xt_raw[:])
            t_wx0_raw = constp.tile([128, 256], f32)
            nc.sync.dma_start(out=t_wx0_raw[:], in_=d_wx0[:])
            t_wx0 = constp.tile([128, 256], f32r)
            nc.vector.tensor_copy(out=t_wx0[:], in_=t_wx0_raw[:])
            t_webb_raw = constp.tile([33, 256], f32)
            nc.sync.dma_start(out=t_webb_raw[:], in_=d_webb[:])
            t_webb = constp.tile([33, 256], f32r)
            nc.vector.tensor_copy(out=t_webb[:], in_=t_webb_raw[:])
            t_iota = constp.tile([128, W], bf16)
            nc.sync.dma_start(out=t_iota[:], in_=d_iota[:])
            t_ones = constp.tile([1, SUP], f32)
            nc.vector.memset(t_ones[:], 1.0)

            for s in range(NSUP):
                t_attr = attrp.tile([128, CPS * 32], bf16)
                nc.sync.dma_start(
                    out=t_attr[:], in_=d_attr[:, s * CPS * 32 : (s + 1) * CPS * 32]
                )
                ps_a = psaggp.tile([32, SUP], f32)
                for w in range(WPS):
                    for k in range(K):
                        ci = w * K + k
                        gci = s * CPS + ci
                        oh = ohp.tile([128, W], bf16)
                        nc.vector.tensor_scalar(
                            oh[:],
                            t_iota[:],
                            t_idx[:, gci : gci + 1],
                            None,
                            op0=mybir.AluOpType.is_equal,
                        )
                        nc.tensor.matmul(
                            out=ps_a[:, w * W : (w + 1) * W],
                            lhsT=t_attr[:, ci * 32 : (ci + 1) * 32],
                            rhs=oh[:],
                            start=(k == 0),
                            stop=(k == K - 1),
                        )
                t_agg = aggp.tile([33, SUP], f32r)
                nc.vector.tensor_copy(out=t_agg[0:32, :], in_=ps_a[:])
                nc.vector.tensor_copy(out=t_agg[32:33, :], in_=t_ones[:])
                for j in range(4):
                    ps_o = psoutp.tile([128, 256], f32)
                    nc.tensor.matmul(
                        out=ps_o[:],
                        lhsT=t_agg[:, j * 128 : (j + 1) * 128],
                        rhs=t_webb[:],
                        start=True,
                        stop=False,
                        skip_group_check=True,
                    )
                    nc.tensor.matmul(
                        out=ps_o[:],
                        lhsT=t_xt[:, s * SUP + j * 128 : s * SUP + (j + 1) * 128],
                        rhs=t_wx0[:],
                        start=False,
                        stop=True,
                        skip_group_check=True,
                    )
                    t_out = outp.tile([128, 256], f32)
                    nc.scalar.activation(
                        out=t_out[:], in_=ps_o[:], func=mybir.ActivationFunctionType.Relu
                    )
                    nc.sync.dma_start(
                        out=d_out[s * SUP + j * 128 : s * SUP + (j + 1) * 128, :],
                        in_=t_out[:],
                    )
    nc.compile()
    return nc


def kernel(x, edge_index, edge_attr, Wx, bx, We, be):
    x = np.asarray(x, dtype=np.float32)
    edge_attr = np.asarray(edge_attr, dtype=np.float32)
    Wx = np.asarray(Wx, dtype=np.float32)
    bx = np.asarray(bx, dtype=np.float32)
    We = np.asarray(We, dtype=np.float32)
    be = np.asarray(be, dtype=np.float32)
    src = np.asarray(edge_index[0], dtype=np.int64)

    # ---- host-side edge bucketing (core -> 64-node window -> 128-edge chunks)
    core = src // NPC                       # 0..7
    rel = src - core * NPC                  # 0..12499
    win = rel // W                          # 0..195
    within = (rel - win * W).astype(np.float32)  # 0..63
    bucket = core * WIN_PER_CORE + win      # global (core, window) id
    nbuckets = NCORES * WIN_PER_CORE
    counts = np.bincount(bucket, minlength=nbuckets)
    K = max(1, int(-(-counts.max() // 128)))   # chunks per window (uniform)
    EPW = 128 * K                              # padded edges per window

    order = np.argsort(bucket, kind="stable")
    sbucket = bucket[order]
    starts = np.zeros(nbuckets + 1, dtype=np.int64)
    starts[1:] = np.cumsum(counts)
    pos = np.arange(E, dtype=np.int64) - starts[sbucket]
    dest = sbucket * EPW + pos

    idx_pad = np.full(nbuckets * EPW, W, dtype=np.float32)  # W==64 never matches iota
    idx_pad[dest] = within[order]
    attr_pad = np.zeros((nbuckets * EPW, FE), dtype=BF16)
    attr_pad[dest] = edge_attr[order].astype(BF16)

    C = WIN_PER_CORE * K
    # ---- per-core input maps
    wx0 = np.zeros((128, 256), dtype=np.float32)
    wx0[:, 0:128] = Wx
    webb = np.zeros((33, 256), dtype=np.float32)
    webb[0:32, 128:256] = We
    webb[32, 0:128] = bx
    webb[32, 128:256] = be
    iota = np.broadcast_to(
        np.arange(W, dtype=np.float32), (128, W)
    ).astype(BF16).copy()

    in_maps = []
    for c in range(NCORES):
        a = attr_pad[c * C * 128 : (c + 1) * C * 128]          # [C*128, 32] bf16
        attr_slab = np.ascontiguousarray(
            a.reshape(C, 128, FE).transpose(1, 0, 2).reshape(128, C * FE)
        )
        idxT = np.ascontiguousarray(
            idx_pad[c * C * 128 : (c + 1) * C * 128].reshape(C, 128).T
        )
        xpad = np.zeros((NPC_PAD, FX), dtype=np.float32)
        xpad[:NPC] = x[c * NPC : (c + 1) * NPC]
        xT = np.ascontiguousarray(xpad.T)                       # [128, NPC_PAD]
        in_maps.append(
            {
                "attr": attr_slab,
                "idx": idxT,
                "xt": xT,
                "wx0": wx0,
                "webb": webb,
                "iota": iota,
            }
        )

    if K not in _program_cache:
        _program_cache[K] = _build_program(K)
    nc = _program_cache[K]

    res = run_bass_kernel_spmd(nc, in_maps, core_ids=list(range(NCORES)))
    global last_results
    last_results = res
    out = np.concatenate([res.results[c]["out"][:NPC] for c in range(NCORES)], axis=0)
    return out


# revision 8
# speedup vs baseline: 1.9348x; 1.9348x over previous
"""GNN message-passing kernel for 8 Trainium2 NeuronCores.

Computes: relu(concat([x @ Wx + bx, segment_sum(edge_attr, src) @ We + be], axis=1))

Strategy (graph-parallel, per the sharding hint):
  - Nodes are sharded 8 ways (12500 per core); edges are bucketed by the
    core that owns their source node, so the segment-sum is core-local.
  - On-device segment-sum: edges are host-bucketed into 64-node windows
    (padded to K chunks of 128 edges each); each chunk contributes via a
    one-hot matmul on the PE (stationary = edge_attr chunk [128,32] bf16,
    moving = one-hot [128,64] bf16), accumulating aggT [32, nodes] in PSUM.
    One-hots for a whole window are built in a single DVE is_equal over
    [128, K*64] comparing an iota pattern against the stride-0-broadcast
    edge indices.
  - Epilogue per 128 nodes: two bf16 matmuls with N=256 fold both linears,
    the bias add and the concat into one PSUM tile ([Wx | 0] and
    [[0 | We]; [bx | be]] with a ones row on aggT); ACT applies relu
    straight out of PSUM.
"""

import sys

sys.path.insert(0, "/opt/trn_rl_repo")

import numpy as np
import ml_dtypes

from concourse import mybir, bacc
import concourse.tile as tile
from concourse.bass_utils import run_bass_kernel_spmd

# Problem constants (hardcoded per the nn_NodeCentric spec)
N = 100_000
E = 1_600_000
FE = 32
FX = 128
OX = 128
OE = 128
NCORES = 8
NPC = N // NCORES          # nodes per core = 12500
W = 64                     # node window (one-hot moving width)
SUP = 512                  # supertile = one PSUM bank of aggT columns
NSUP = (NPC + SUP - 1) // SUP          # 25
NPC_PAD = NSUP * SUP                   # 12800
WIN_PER_CORE = NPC_PAD // W            # 200
WPS = SUP // W                         # 8 windows per supertile

BF16 = ml_dtypes.bfloat16

_program_cache: dict[int, object] = {}
last_results = None  # BassKernelResults of the most recent run (for test harness)


def _build_program(K: int):
    """Build the (identical-across-cores) Bass program for K chunks/window."""
    C = WIN_PER_CORE * K        # chunks per core
    CPS = WPS * K               # chunks per supertile

    nc = bacc.Bacc("TRN2", target_bir_lowering=False, debug=False)
    f32 = mybir.dt.float32
    bf16 = mybir.dt.bfloat16

    d_attr = nc.dram_tensor("attr", [128, C * 32], bf16, kind="ExternalInput")
    d_idx = nc.dram_tensor("idx", [128, C], bf16, kind="ExternalInput")
    d_xt = nc.dram_tensor("xt", [128, NPC_PAD], bf16, kind="ExternalInput")
    d_wx0 = nc.dram_tensor("wx0", [128, 256], bf16, kind="ExternalInput")
    d_webb = nc.dram_tensor("webb", [33, 256], bf16, kind="ExternalInput")
    d_iota = nc.dram_tensor("iota", [128, K * W], bf16, kind="ExternalInput")
    d_out = nc.dram_tensor("out", [NPC_PAD, 256], f32, kind="ExternalOutput")

    with tile.TileContext(nc) as tc:
        with (
            tc.tile_pool(name="const", bufs=1) as constp,
            tc.tile_pool(name="attr", bufs=3) as attrp,
            tc.tile_pool(name="oh", bufs=4) as ohp,
            tc.tile_pool(name="agg", bufs=2) as aggp,
            tc.tile_pool(name="outs", bufs=3) as outp,
            tc.tile_pool(name="psagg", bufs=2, space="PSUM") as psaggp,
            tc.tile_pool(name="psout", bufs=2, space="PSUM") as psoutp,
        ):
            t_idx = constp.tile([128, C], bf16)
            nc.sync.dma_start(out=t_idx[:], in_=d_idx[:])
            t_xt = constp.tile([128, NPC_PAD], bf16)
            nc.sync.dma_start(out=t_xt[:], in_=d_xt[:])
            t_wx0 = constp.tile([128, 256], bf16)
            nc.sync.dma_start(out=t_wx0[:], in_=d_wx0[:])
            t_webb = constp.tile([33, 256], bf16)
            nc.sync.dma_start(out=t_webb[:], in_=d_webb[:])
            t_iota = constp.tile([128, K * W], bf16)
            nc.sync.dma_start(out=t_iota[:], in_=d_iota[:])
            t_ones = constp.tile([1, SUP], bf16)
            nc.vector.memset(t_ones[:], 1.0)

            for s in range(NSUP):
                t_attr = attrp.tile([128, CPS * 32], bf16)
                nc.sync.dma_start(
                    out=t_attr[:], in_=d_attr[:, s * CPS * 32 : (s + 1) * CPS * 32]
                )
                ps_a = psaggp.tile([32, SUP], f32)
                for w in range(WPS):
                    cw = s * CPS + w * K    # first chunk of this window (global)
                    oh = ohp.tile([128, K * W], bf16)
                    nc.vector.tensor_tensor(
                        out=oh[:].rearrange("p (k f) -> p k f", f=W),
                        in0=t_iota[:].rearrange("p (k f) -> p k f", f=W),
                        in1=t_idx[:, cw : cw + K].to_broadcast([128, K, W]),
                        op=mybir.AluOpType.is_equal,
                    )
                    for k in range(K):
                        ci = w * K + k
                        nc.tensor.matmul(
                            out=ps_a[:, w * W : (w + 1) * W],
                            lhsT=t_attr[:, ci * 32 : (ci + 1) * 32],
                            rhs=oh[:, k * W : (k + 1) * W],
                            start=(k == 0),
                            stop=(k == K - 1),
                        )
                t_agg = aggp.tile([33, SUP], bf16)
                nc.vector.tensor_copy(out=t_agg[0:32, :], in_=ps_a[:])
                nc.vector.tensor_copy(out=t_agg[32:33, :], in_=t_ones[:])
                for j in range(4):
                    ps_o = psoutp.tile([128, 256], f32)
                    nc.tensor.matmul(
                        out=ps_o[:],
                        lhsT=t_agg[:, j * 128 : (j + 1) * 128],
                        rhs=t_webb[:],
                        start=True,
                        stop=False,
                        skip_group_check=True,
                    )
                    nc.tensor.matmul(
                        out=ps_o[:],
                        lhsT=t_xt[:, s * SUP + j * 128 : s * SUP + (j + 1) * 128],
                        rhs=t_wx0[:],
                        start=False,
                        stop=True,
                        skip_group_check=True,
                    )
                    t_out = outp.tile([128, 256], f32)
                    nc.scalar.activation(
                        out=t_out[:], in_=ps_o[:], func=mybir.ActivationFunctionType.Relu
                    )
                    nc.sync.dma_start(
                        out=d_out[s * SUP + j * 128 : s * SUP + (j + 1) * 128, :],
                        in_=t_out[:],
                    )
    nc.compile()
    return nc


def kernel(x, edge_index, edge_attr, Wx, bx, We, be):
    x = np.asarray(x, dtype=np.float32)
    edge_attr = np.asarray(edge_attr, dtype=np.float32)
    Wx = np.asarray(Wx, dtype=np.float32)
    bx = np.asarray(bx, dtype=np.float32)
    We = np.asarray(We, dtype=np.float32)
    be = np.asarray(be, dtype=np.float32)
    src = np.asarray(edge_index[0], dtype=np.int64)

    # ---- host-side edge bucketing (core -> 64-node window -> 128-edge chunks)
    core = src // NPC                       # 0..7
    rel = src - core * NPC                  # 0..12499
    win = rel // W                          # 0..195
    within = (rel - win * W).astype(np.float32)  # 0..63
    bucket = core * WIN_PER_CORE + win      # global (core, window) id
    nbuckets = NCORES * WIN_PER_CORE
    counts = np.bincount(bucket, minlength=nbuckets)
    K = max(1, int(-(-counts.max() // 128)))   # chunks per window (uniform)
    EPW = 128 * K                              # padded edges per window

    order = np.argsort(bucket, kind="stable")
    sbucket = bucket[order]
    starts = np.zeros(nbuckets + 1, dtype=np.int64)
    starts[1:] = np.cumsum(counts)
    pos = np.arange(E, dtype=np.int64) - starts[sbucket]
    dest = sbucket * EPW + pos

    idx_pad = np.full(nbuckets * EPW, W, dtype=np.float32)  # W==64 never matches iota
    idx_pad[dest] = within[order]
    attr_pad = np.zeros((nbuckets * EPW, FE), dtype=BF16)
    attr_pad[dest] = edge_attr[order].astype(BF16)

    C = WIN_PER_CORE * K
    # ---- per-core input maps
    wx0 = np.zeros((128, 256), dtype=BF16)
    wx0[:, 0:128] = Wx.astype(BF16)
    webb = np.zeros((33, 256), dtype=BF16)
    webb[0:32, 128:256] = We.astype(BF16)
    webb[32, 0:128] = bx.astype(BF16)
    webb[32, 128:256] = be.astype(BF16)
    iota = np.broadcast_to(
        np.tile(np.arange(W, dtype=np.float32), K), (128, K * W)
    ).astype(BF16).copy()

    in_maps = []
    for c in range(NCORES):
        a = attr_pad[c * C * 128 : (c + 1) * C * 128]          # [C*128, 32] bf16
        attr_slab = np.ascontiguousarray(
            a.reshape(C, 128, FE).transpose(1, 0, 2).reshape(128, C * FE)
        )
        idxT = np.ascontiguousarray(
            idx_pad[c * C * 128 : (c + 1) * C * 128].reshape(C, 128).T
        ).astype(BF16)
        xpad = np.zeros((NPC_PAD, FX), dtype=np.float32)
        xpad[:NPC] = x[c * NPC : (c + 1) * NPC]
        xT = np.ascontiguousarray(xpad.T.astype(BF16))          # [128, NPC_PAD]
        in_maps.append(
            {
                "attr": attr_slab,
                "idx": idxT,
                "xt": xT,
                "wx0": wx0,
                "webb": webb,
                "iota": iota,
            }
        )

    if K not in _program_cache:
        _program_cache[K] = _build_program(K)
    nc = _program_cache[K]

    res = run_bass_kernel_spmd(nc, in_maps, core_ids=list(range(NCORES)))
    global last_results
    last_results = res
    out = np.concatenate([res.results[c]["out"][:NPC] for c in range(NCORES)], axis=0)
    return out
